# revision 93
# baseline (speedup 1.0000x reference)
"""Multi-head self-attention (B=16, N=784, D=768, H=12) on 8 trn2 cores.

Sharding: pure data-parallel over batch (2 batches per core, no collectives).
All matmuls bf16 with fp32 PSUM accumulation.

v2 over baseline: score matmuls for a head pair (which share one qkt tile —
head 2p at partitions 0:64, head 2p+1 at 64:128) are emitted back-to-back
into two separate PSUM tiles. They use disjoint PE row groups
(tile_position (0,0)/(64,0)) so the hardware runs them concurrently,
doubling effective PE-array utilization for the scores phase (K=64 would
otherwise use half the array). PV keeps the ones-augmented V slab (M=65,
softmax denominator in row 64); PV for the previous pair is fed from a work
queue, a few matmuls per tj slot, using one [65, 1024] PSUM tile at a time.
NOTE: col-tiled concurrent matmul pairs (two PE writers into one PSUM bank)
fault the device — do not re-introduce them.

PSUM budget (8 banks): scores ring 2 x [128,1024] (4 banks) + aux ring
1 x [128,1024] (2, projection accumulators, emitted atomically per chunk)
+ PV accumulator 1 x [65,1024] (2).

Loop order everywhere keeps the col-group innermost so consecutive MMs share
the stationary operand (saves the PE weight-swap drain bubble).
"""

from collections import deque
from contextlib import ExitStack

import ml_dtypes
import numpy as np

import concourse.mybir as mybir
import concourse.tile as tile
from concourse import bacc
from concourse.bass_utils import run_bass_kernel_spmd

dt = mybir.dt
AF = mybir.ActivationFunctionType

B, N, D = 16, 784, 768
H, HD = 12, 64
F3 = 3 * D  # 2304
N_CORES = 8
BPC = B // N_CORES  # batches per core

T_CHUNKS = [(i * 128, min(128, N - i * 128)) for i in range((N + 127) // 128)]
NT = len(T_CHUNKS)  # 7
ND = D // 128  # 6
COLS_N = [(0, 512), (512, N - 512)]
COLS_D = [(0, 512), (512, D - 512)]

BF = dt.bfloat16


class _Slab:
    """View adapter: slab[di][:, c0:c1] -> big_tile[:, di*pitch+c0 : di*pitch+c1].

    Lets one [128, ND*pitch] SBUF tile (loaded by a few multi-block DMAs)
    stand in for a list of ND per-row-block tiles.
    """

    def __init__(self, tile, pitch):
        self.tile, self.pitch = tile, pitch

    def __getitem__(self, di):
        return _SlabRow(self.tile, di * self.pitch)


class _SlabRow:
    def __init__(self, tile, base):
        self.tile, self.base = tile, base

    def __getitem__(self, key):
        ps, cs = key
        return self.tile[ps, self.base + cs.start:self.base + cs.stop]


def _setup_consts(nc, P, aps, st):
    """Emit input DMAs in consumption-priority order across the two
    HW-DGE queues (sync, scalar) plus the gpsimd SW queue.

    Each dma_start costs ~600ns of sequencer issue time, so loads are
    coalesced: xt/wq/wo are single [128, ND*pitch] slabs filled by
    multi-row-block DMAs (3D access patterns), column-sliced to match
    the downstream consumption order (fi 0,6,1,7 -> V -> fi 2..5,8..11).
    Sub-tile dependency tracking lets each matmul wait only for the DMA
    piece covering the columns it reads.
    """
    xt0_t = P["xt"].tile([128, ND * N], BF, name="xt0", tag="xt")
    wq_t = P["wq"].tile([128, ND * F3], BF, name="wq", tag="wq")
    wo_t = P["wo"].tile([128, ND * D], BF, name="wo", tag="wo")
    xt0 = _Slab(xt0_t, N)
    wq16 = _Slab(wq_t, F3)
    wo16 = _Slab(wo_t, D)
    xsv = aps["xs"].rearrange("b (k p) n -> b p k n", k=ND)
    wqv = aps["wqkv"].rearrange("(k p) c -> p k c", k=ND)
    wov = aps["wo"].rearrange("(k p) c -> p k c", k=ND)
    xtv = xt0_t[:].rearrange("p (k n) -> p k n", k=ND)
    wqt = wq_t[:].rearrange("p (k c) -> p k c", k=ND)
    wot = wo_t[:].rearrange("p (k c) -> p k c", k=ND)

    def wq_dma(eng, c0, cw):
        eng.dma_start(wqt[:, :, c0:c0 + cw], wqv[:, :, c0:c0 + cw])

    # sync: x row-blocks 0-2, fi=0 cols, outer V col group
    nc.sync.dma_start(xtv[:, 0:3, :], xsv[0, :, 0:3, :])
    wq_dma(nc.sync, 0, 128)                              # fi=0
    wq_dma(nc.sync, 2 * D, COLS_D[0][1])                 # V cols 0:512
    # scalar: x row-blocks 3-5, fi=6, inner V col group, fi=7
    nc.scalar.dma_start(xtv[:, 3:6, :], xsv[0, :, 3:6, :])
    wq_dma(nc.scalar, 768, 128)                          # fi=6
    wq_dma(nc.scalar, 2 * D + COLS_D[1][0], COLS_D[1][1])  # V cols 512:768
    wq_dma(nc.scalar, 896, 128)                          # fi=7
    # gpsimd: biases + fi=1 early; late-consumed pieces after (b1's x,
    # Q/K cols fi 2-5/8-11 — consumed in c1 — and wo)
    bqc = P["konst"].tile([128, F3 // 128], dt.float32, name="bqc")
    nc.gpsimd.dma_start(bqc[:], aps["bqc"][:])
    wq_dma(nc.gpsimd, 128, 128)                          # fi=1
    bqv_r = P["konst"].tile([1, D], dt.float32, name="bqv_r")
    nc.gpsimd.dma_start(bqv_r[:], aps["bqv"][:])
    bqv_bc = P["konst"].tile([128, D], dt.float32, name="bqv_bc")
    nc.gpsimd.partition_broadcast(bqv_bc[:], bqv_r[0:1, :])
    bo_r = P["konst"].tile([1, D], dt.float32, name="bo_r")
    nc.gpsimd.dma_start(bo_r[:], aps["bo"][:])
    bo_bc = P["konst"].tile([128, D], dt.float32, name="bo_bc")
    nc.gpsimd.partition_broadcast(bo_bc[:], bo_r[0:1, :])
    st["xt0"] = xt0
    st["late_dmas"] = lambda: (
        wq_dma(nc.gpsimd, 256, 512),    # fi=2..5
        wq_dma(nc.gpsimd, 1024, 512),   # fi=8..11
        nc.gpsimd.dma_start(wot[:], wov[:]),
    )
    return dict(bqc=bqc, bqv_bc=bqv_bc, bo_bc=bo_bc, wq16=wq16, wo16=wo16)


def _gen_a(nc, P, C, aps, b, st):
    if b == 0:
        yield  # tiles already loaded by _setup_consts
        return
    xt_t = P["xt"].tile([128, ND * N], BF, name=f"xt{b}", tag="xt")
    xtv = xt_t[:].rearrange("p (k n) -> p k n", k=ND)
    xsv = aps["xs"].rearrange("b (k p) n -> b p k n", k=ND)
    nc.gpsimd.dma_start(xtv[:], xsv[b])
    st["late_dmas"]()
    yield
    st[f"xt{b}"] = _Slab(xt_t, N)


def _aux_ps(nc, P, st):
    """Projection PSUM allocator returning a slicing closure. Before
    attention PV starts (first ~8 tiles of batch 0), alternate between
    the aux pool and the then-idle PV pool (as two tiles matching its
    tag sizes) so drains double-buffer instead of serializing the PE."""
    n = st.get("auxn", 0)
    st["auxn"] = n + 1
    if n < 8 and n % 2 == 1:
        a = P["ps_ot"].tile([128, 512], dt.float32, name="pj_a", tag="ot0")
        b = P["ps_ot"].tile([128, 272], dt.float32, name="pj_b", tag="ot1")

        def sl(ps, c0, cw):
            if c0 >= 512:
                return b[ps, c0 - 512:c0 - 512 + cw]
            return a[ps, c0:c0 + cw]
        return sl
    t = P["ps_aux"].tile([128, 1024], dt.float32, name="pj_ps", tag="aux")

    def sl(ps, c0, cw):
        return t[ps, c0:c0 + cw]
    return sl


def _gen_b_qk(nc, P, C, b, st):
    """Q,K transposed layout: 12 tiles [128, 784]. di outer / cg inner."""
    xt16 = st[f"xt{b}"]
    qkt16 = [None] * 12
    st[f"qkt{b}"] = qkt16
    for fi in [0, 6, 1, 7, 2, 8, 3, 9, 4, 10, 5, 11]:
        qk_ps = _aux_ps(nc, P, st)
        for di in range(ND):
            for (c0, cw) in reversed(COLS_N):
                nc.tensor.matmul(
                    qk_ps(slice(None), c0, cw),
                    C["wq16"][di][:, fi * 128:(fi + 1) * 128],
                    xt16[di][:, c0:c0 + cw],
                    start=(di == 0), stop=(di == ND - 1))
            if di == 2:
                yield  # ~1us unit boundary mid-accumulation
        q = P["qkt"].tile([128, N], BF, name=f"qkt{b}_{fi}", tag="qkt")
        # drain bank B then A (matmul fill order is B, A) so the next aux
        # tile's first matmul only waits for the B-bank drain
        for (c0, cw) in reversed(COLS_N):
            nc.vector.tensor_scalar_add(q[:, c0:c0 + cw], qk_ps(slice(None), c0, cw),
                                        C["bqc"][0:128, fi:fi + 1])
        qkt16[fi] = q
        yield


def _gen_b_v(nc, P, C, b, st):
    """V slab [t, 12, 65]: 65th column is 1.0 (softmax denominator trick)."""
    xt16 = st[f"xt{b}"]
    vt16 = []
    st[f"vt{b}"] = vt16
    for (t0, p), ti in zip(T_CHUNKS, range(NT)):
        v_ps = _aux_ps(nc, P, st)
        for di in range(ND):
            for (c0, cw) in reversed(COLS_D):
                nc.tensor.matmul(
                    v_ps(slice(0, p), c0, cw),
                    xt16[di][:, t0:t0 + p],
                    C["wq16"][di][:, 2 * D + c0:2 * D + c0 + cw],
                    start=(di == 0), stop=(di == ND - 1))
            if di == 2:
                yield
        vt = P["vt"].tile([128, H, HD + 1], BF, name=f"vt{b}_{ti}", tag="vt")
        # drain per bank (features 512:768 = heads 8-11 fill first)
        for (h0, h1) in ((8, 12), (0, 8)):
            nc.vector.tensor_tensor(
                vt[0:p, h0:h1, 0:HD],
                v_ps(slice(0, p), h0 * HD, (h1 - h0) * HD).rearrange(
                    "p (h d) -> p h d", h=h1 - h0),
                C["bqv_bc"][0:p, h0 * HD:h1 * HD].rearrange("p (h d) -> p h d", h=h1 - h0),
                mybir.AluOpType.add)
        nc.vector.memset(vt[0:p, :, HD:HD + 1], 1.0)
        vt16.append(vt)
        yield


def _pv_thunks(nc, P, b, st, p):
    """Work queue for pair p's PV + normalize (two heads, sequential).

    Per head: two PSUM accumulators (one per col group, 1 bank each), 14
    accumulating MMs tj-outer / cg-inner (consecutive MMs share the vt
    stationary; ex consumption is tj-monotone), then drain + normalize
    into ot16[p] rows hh*64. Row 64 of the accumulators is the softmax
    denominator (ones column of the V slab).
    """
    vt16, ex = st[f"vt{b}"], st[f"ex{b}_{p}"]
    work = deque()
    holder = {}

    def _alloc(hh):
        def f():
            holder[0] = P["ps_ot"].tile([HD + 1, 512], dt.float32,
                                        name=f"pv0_{p}_{hh}", tag="ot0")
            holder[1] = P["ps_ot"].tile([HD + 1, COLS_N[1][1]], dt.float32,
                                        name=f"pv1_{p}_{hh}", tag="ot1")
        return f

    def _mms(hh, tjs):
        h = 2 * p + hh

        def f():
            for tj in tjs:
                (t0, pj) = T_CHUNKS[tj]
                for ci, (c0, cw) in reversed(list(enumerate(COLS_N))):
                    nc.tensor.matmul(
                        holder[ci][0:HD + 1, 0:cw],
                        vt16[tj][0:pj, h, 0:HD + 1],
                        ex[2 * tj + hh][0:pj, c0:c0 + cw],
                        start=(tj == 0), stop=(tj == NT - 1))
        return f

    fstate = {}

    def _finish_a(hh):
        """Drain holder to SBUF, reciprocal of the denominator row, and
        gpsimd broadcast. The multiply is deferred to _finish_b so head
        hh=1's vector work can overlap head hh=0's gpsimd broadcast."""
        def f():
            osb = P["osb"].tile([HD + 1, N], dt.float32, name="osb65", tag="osb")
            for ci, (c0, cw) in enumerate(COLS_N):
                nc.vector.tensor_copy(osb[0:HD + 1, c0:c0 + cw],
                                      holder[ci][0:HD + 1, 0:cw])
            srow = P["recp"].tile([1, N], dt.float32, name="srow", tag="srow")
            nc.vector.tensor_copy(srow[0:1, :], osb[HD:HD + 1, :])
            rec = P["recp"].tile([1, N], dt.float32, name="rec", tag="rec")
            nc.vector.reciprocal_approx_fast(rec[0:1, :], srow[0:1, :])
            brec = P["brec"].tile([HD, N], dt.float32, name="brec", tag="brec")
            nc.gpsimd.partition_broadcast(brec[0:HD, :], rec[0:1, :])
            fstate[hh] = (osb, brec)
        return f

    def _finish_b(hh):
        def f():
            if hh == 0:
                ot = P["ot"].tile([128, N], BF, name=f"ot{b}_{p}", tag="ot")
                st[f"ot{b}"][p] = ot
            ot = st[f"ot{b}"][p]
            osb, brec = fstate[hh]
            ro = hh * HD
            nc.vector.tensor_mul(ot[ro:ro + HD, :], osb[0:HD, :], brec[0:HD, :])
        return f

    for hh in range(2):
        work.append(_alloc(hh))
        work.append(_mms(hh, range(0, 2)))
        work.append(_mms(hh, range(2, 4)))
        work.append(_mms(hh, range(4, 6)))
        work.append(_mms(hh, range(6, NT)))
        work.append(_finish_a(hh))
    work.append(_finish_b(0))
    work.append(_finish_b(1))
    return work


def _pop(work, k):
    n = 0
    while work and n < k:
        work.popleft()()
        n += 1
    return n


def _gen_c(nc, P, C, b, st):
    """Attention for batch b. Yields once per (pair, tj)."""
    qkt16 = st[f"qkt{b}"]
    st[f"ot{b}"] = [None] * 6
    pv_work = deque()
    for p in range(6):
        ex = [P["ex"].tile([128, N], BF, name="ex", tag="ex") for _ in range(2 * NT)]
        st[f"ex{b}_{p}"] = ex
        if p >= 1:
            assert not pv_work
            pv_work = _pv_thunks(nc, P, b, st, p - 1)
        for (t0, pj), tj in zip(T_CHUNKS, range(NT)):
            # prev-pair PV (and the outer loop's filler, via yield) go in
            # front of the scores so the PE reaches the next score pair
            # after ACT has drained the previous one. At a pair's FIRST
            # iteration, defer 2 of the 3 pops until after the exps: the
            # measured ACT gaps sit at pair boundaries, where the full
            # pop batch + filler otherwise push the scores ~1us past the
            # PSUM-ring-free point.
            yield
            qt, kt = qkt16[p], qkt16[6 + p]
            sc_a = P["ps_sc"].tile([128, 1024], dt.float32, name="sc_a", tag="sc")
            sc_b = P["ps_sc"].tile([128, 1024], dt.float32, name="sc_b", tag="sc")
            # col-outer / head-inner: consecutive matmuls sit on disjoint
            # PE row quadrants (and different PSUM banks), so each pair
            # runs concurrently — issue order head-outer would serialize
            # the two heads' same-quadrant matmuls instead.
            for (c0, cw) in reversed(COLS_N):
                for hh, sc in ((0, sc_a), (1, sc_b)):
                    ro = hh * HD
                    nc.tensor.matmul(
                        sc[0:pj, c0:c0 + cw],
                        kt[ro:ro + HD, t0:t0 + pj],
                        qt[ro:ro + HD, c0:c0 + cw],
                        start=True, stop=True)
            for hh, sc in ((0, sc_a), (1, sc_b)):
                nc.scalar.activation(ex[2 * tj + hh][0:pj, :], sc[0:pj, 0:N],
                                     AF.Exp, scale=float(HD) ** -0.5)
            _pop(pv_work, 3)
        _pop(pv_work, 99)  # finish prev pair's PV before ex tiles rotate far
    # epilogue: last pair's PV, yielding so the caller can interleave the
    # output projection's early chunks
    pv_work = _pv_thunks(nc, P, b, st, 5)
    while pv_work:
        _pop(pv_work, 3)
        yield


def _gen_d(nc, P, C, aps, b, st):
    """Full output projection for batch b (used for b=0, scheduled during
    c1 where all of b0's ot tiles are ready). 14 yields: mms / drain per
    chunk, on the 1-buf aux pool."""
    ot16 = st[f"ot{b}"]
    tiles = [None] * NT

    def _mms(ti):
        (t0, p) = T_CHUNKS[ti]
        tiles[ti] = P["ps_aux"].tile([128, 1024], dt.float32, name="y_ps", tag="aux")
        y_ps = tiles[ti]
        for oi in range(ND):
            for (c0, cw) in reversed(COLS_D):
                nc.tensor.matmul(
                    y_ps[0:p, c0:c0 + cw],
                    ot16[oi][:, t0:t0 + p],
                    C["wo16"][oi][:, c0:c0 + cw],
                    start=(oi == 0), stop=(oi == ND - 1))

    def _drain(ti):
        (t0, p) = T_CHUNKS[ti]
        y16 = P["yout"].tile([128, D], BF, name="y16", tag="y16")
        for (c0, cw) in reversed(COLS_D):
            nc.vector.tensor_add(y16[0:p, c0:c0 + cw], tiles[ti][0:p, c0:c0 + cw],
                                 C["bo_bc"][0:p, c0:c0 + cw])
        # all y stores ride the sync queue: a dma_start costs ~667ns of
        # sequencer time, and the scalar sequencer also dispatches the
        # exp ACTIVATEs that pace the attention phase
        qeng = nc.sync
        qeng.dma_start(aps["ys"][b, t0:t0 + p, :], y16[0:p, :])

    for ti in range(NT):
        _mms(ti)
        yield
        _drain(ti)
        yield


def _gen_d1P(nc, P, C, aps, st):
    """Batch-1 projection, partial stage: accumulate oi 0:3 (head pairs
    0-2, ready mid-c1) into aux PSUM, drain (+bias) to a bf16 partial.
    7 yields."""
    ot16 = st["ot1"]
    parts = st.setdefault("d1parts", [None] * NT)
    for ti in range(NT):
        (t0, p) = T_CHUNKS[ti]
        y_ps = P["ps_aux"].tile([128, 1024], dt.float32, name="yp_ps", tag="aux")
        for oi in range(3):
            for (c0, cw) in reversed(COLS_D):
                nc.tensor.matmul(
                    y_ps[0:p, c0:c0 + cw],
                    ot16[oi][:, t0:t0 + p],
                    C["wo16"][oi][:, c0:c0 + cw],
                    start=(oi == 0), stop=(oi == 2))
        part = P["part"].tile([128, D], BF, name="ypart", tag="part")
        for (c0, cw) in reversed(COLS_D):
            nc.vector.tensor_add(part[0:p, c0:c0 + cw], y_ps[0:p, c0:c0 + cw],
                                 C["bo_bc"][0:p, c0:c0 + cw])
        parts[ti] = part
        yield


def _gen_d1F(nc, P, C, aps, st):
    """Batch-1 projection, final stage: oi 3:6 into sc PSUM (2 bufs) then
    y = psum + partial. Split per chunk into F1 (oi 3:5 — pairs 3,4) and
    F2 (oi 5 + drain) so F1 can be pulled before pair 5 finishes."""
    ot16 = st["ot1"]
    parts = st["d1parts"]
    tiles = [None] * NT

    def _f1(ti):
        (t0, p) = T_CHUNKS[ti]
        tiles[ti] = P["ps_sc"].tile([128, 1024], dt.float32, name="yf_ps", tag="sc")
        y_ps = tiles[ti]
        for oi in (3, 4):
            for (c0, cw) in reversed(COLS_D):
                nc.tensor.matmul(
                    y_ps[0:p, c0:c0 + cw],
                    ot16[oi][:, t0:t0 + p],
                    C["wo16"][oi][:, c0:c0 + cw],
                    start=(oi == 3), stop=False)

    def _f2(ti):
        (t0, p) = T_CHUNKS[ti]
        y_ps = tiles[ti]
        for (c0, cw) in reversed(COLS_D):
            nc.tensor.matmul(
                y_ps[0:p, c0:c0 + cw],
                ot16[5][:, t0:t0 + p],
                C["wo16"][5][:, c0:c0 + cw],
                start=False, stop=True)
        y16 = P["yout"].tile([128, D], BF, name="y16", tag="y16")
        for (c0, cw) in reversed(COLS_D):
            nc.vector.tensor_add(y16[0:p, c0:c0 + cw], y_ps[0:p, c0:c0 + cw],
                                 parts[ti][0:p, c0:c0 + cw])
        # all y stores ride the sync queue: a dma_start costs ~667ns of
        # sequencer time, and the scalar sequencer also dispatches the
        # exp ACTIVATEs that pace the attention phase
        qeng = nc.sync
        qeng.dma_start(aps["ys"][1, t0:t0 + p, :], y16[0:p, :])

    _f1(0)
    yield
    _f1(1)
    yield
    _f2(0)
    yield
    for ti in range(2, NT):
        _f1(ti)
        yield
        _f2(ti - 1)
        yield
    _f2(NT - 1)


def _exhaust(g):
    for _ in g:
        pass


def _pull(g, k):
    n = 0
    for _ in range(k):
        try:
            next(g)
        except StopIteration:
            break
        n += 1
    return n


POOL_SPECS = [
    ("konst", 1, "SBUF"), ("wq", 1, "SBUF"), ("wo", 1, "SBUF"),
    ("xt", 2, "SBUF"),
    ("qkt", 18, "SBUF"), ("vt", 2 * NT, "SBUF"), ("ex", 20, "SBUF"),
    ("ot", 12, "SBUF"), ("osb", 3, "SBUF"), ("recp", 2, "SBUF"),
    ("brec", 2, "SBUF"),
    ("yout", 2, "SBUF"), ("part", 8, "SBUF"),
    ("ps_sc", 2, "PSUM"), ("ps_aux", 1, "PSUM"), ("ps_ot", 1, "PSUM"),
]


def build():
    nc = bacc.Bacc("TRN2", target_bir_lowering=False, debug=False)

    aps = {
        "xs": nc.dram_tensor("xs", [BPC, D, N], BF, kind="ExternalInput").ap(),
        "wqkv": nc.dram_tensor("wqkv", [D, F3], BF, kind="ExternalInput").ap(),
        "bqc": nc.dram_tensor("bqc", [128, F3 // 128], dt.float32, kind="ExternalInput").ap(),
        "bqv": nc.dram_tensor("bqv", [1, D], dt.float32, kind="ExternalInput").ap(),
        "wo": nc.dram_tensor("wo", [D, D], BF, kind="ExternalInput").ap(),
        "bo": nc.dram_tensor("bo", [1, D], dt.float32, kind="ExternalInput").ap(),
        "ys": nc.dram_tensor("ys", [BPC, N, D], BF, kind="ExternalOutput").ap(),
    }

    with ExitStack() as ctx:
        tc = ctx.enter_context(tile.TileContext(nc))
        P = {}
        for pname, bufs, space in POOL_SPECS:
            P[pname] = ctx.enter_context(
                tc.tile_pool(name=pname, bufs=bufs, space=space))

        st = {}
        C = _setup_consts(nc, P, aps, st)
        import itertools
        def _interleave(gq, gv):
            # qk pair chunks (2 per attention pair) woven with V chunks;
            # fi {0,6,1,7} first to match the DMA piece priority order.
            # Each tile is two ~1us yield-units now, so double each char.
            order = "".join(c * 2 for c in "qqqqvvvvvvvqqqqqqqq")
            for ch in order:
                g = gq if ch == "q" else gv
                try:
                    yield next(g)
                except StopIteration:
                    pass

        f0 = itertools.chain(
            _gen_a(nc, P, C, aps, 0, st),
            _interleave(_gen_b_qk(nc, P, C, 0, st), _gen_b_v(nc, P, C, 0, st)))
        f1 = itertools.chain(
            _gen_a(nc, P, C, aps, 1, st),
            _interleave(_gen_b_qk(nc, P, C, 1, st), _gen_b_v(nc, P, C, 1, st)))
        _pull(f0, 7)  # xt-b0 record + qk fi0, fi6, fi1 (2 units each)
        _pull(f1, 1)  # xt-b1 DMA (queued behind the startup pieces)

        # c0 (batch-0 attention): b0's prep as ~1us units — dense while
        # the V slab is needed (2/yield through i=7), then one unit per
        # yield so filler lumps between the PV pops and the scores stay
        # small and the exps are never pushed late. During c0's
        # epilogue, also drive c1's first yields so batch-1's pair-0
        # scores overlap c0's tail.
        c0 = _gen_c(nc, P, C, 0, st)
        c1 = _gen_c(nc, P, C, 1, st)
        i = 0
        for _ in c0:
            if i >= 42:
                _pull(c1, 1)                 # c1 scores ramp (yields 0-4)
                _pull(f1, 1)                 # v4u2, v5, v6 of b1
            elif i < 8:
                _pull(f0, 2)                 # fi7 + v0-v6 (vt6 by i=7)
            elif i <= 23:
                _pull(f0, 1)                 # fi 2,8,3,9,4,10,5,11 units
            elif i >= 25:
                _pull(f1, 1)                 # b1: fi0,6,1,7 + v0-v4u1
            i += 1
        # c1 (batch-1 attention, yields 5-46): b1's last qk units on odd
        # 5-35, d0 on even 6-32, d1 partials on even 34-46, F1 on 41/43.
        d0 = _gen_d(nc, P, C, aps, 0, st)
        d1p = _gen_d1P(nc, P, C, aps, st)
        d1f = _gen_d1F(nc, P, C, aps, st)
        j = 5
        for _ in c1:
            if j % 2 == 1 and j <= 35:
                _pull(f1, 1)                 # fi 2..11 remaining units
            elif j % 2 == 0 and 6 <= j <= 32:
                _pull(d0, 1)
            elif j % 2 == 0 and j >= 34:
                _pull(d1p, 1)
            elif j in (41, 43):
                _pull(d1f, 1)                # F1 chunks 0,1 (pairs 3,4)
            j += 1
        _exhaust(f0)
        _exhaust(f1)
        _exhaust(d0)
        _exhaust(d1p)
        _exhaust(d1f)

    nc.compile()
    return nc


_NC_CACHE = {}


def _get_nc():
    if "nc" not in _NC_CACHE:
        _NC_CACHE["nc"] = build()
    return _NC_CACHE["nc"]


def make_in_maps(x, Wqkv, bqkv, Wo, bo):
    bf = ml_dtypes.bfloat16
    x = np.asarray(x, dtype=np.float32)
    Wqkv16 = np.ascontiguousarray(np.asarray(Wqkv, np.float32).astype(bf))
    bqkv = np.asarray(bqkv, dtype=np.float32)
    Wo16 = np.ascontiguousarray(np.asarray(Wo, np.float32).astype(bf))
    bo = np.asarray(bo, dtype=np.float32)
    bqc = np.ascontiguousarray(bqkv.reshape(F3 // 128, 128).T)
    bqv = np.ascontiguousarray(bqkv[2 * D:].reshape(1, D))
    bo_r = np.ascontiguousarray(bo.reshape(1, D))
    x16 = np.ascontiguousarray(x.astype(bf).transpose(0, 2, 1))
    in_maps = []
    for c in range(N_CORES):
        in_maps.append({
            "xs": np.ascontiguousarray(x16[c * BPC:(c + 1) * BPC]),
            "wqkv": Wqkv16,
            "bqc": bqc,
            "bqv": bqv,
            "wo": Wo16,
            "bo": bo_r,
        })
    return in_maps


def run(x, Wqkv, bqkv, Wo, bo, trace=False, **kw):
    nc = _get_nc()
    in_maps = make_in_maps(x, Wqkv, bqkv, Wo, bo)
    res = run_bass_kernel_spmd(nc, in_maps, list(range(N_CORES)), trace=trace, **kw)
    out = np.concatenate(
        [np.asarray(res.results[c]["ys"]).astype(np.float32) for c in range(N_CORES)],
        axis=0)
    return out, res


def kernel(x, Wqkv, bqkv, Wo, bo):
    out, _ = run(x, Wqkv, bqkv, Wo, bo)
    return out



# revision 94
# speedup vs baseline: 1.0060x; 1.0060x over previous
"""Multi-head self-attention (B=16, N=784, D=768, H=12) on 8 trn2 cores.

Sharding: pure data-parallel over batch (2 batches per core, no collectives).
All matmuls bf16 with fp32 PSUM accumulation.

v2 over baseline: score matmuls for a head pair (which share one qkt tile —
head 2p at partitions 0:64, head 2p+1 at 64:128) are emitted back-to-back
into two separate PSUM tiles. They use disjoint PE row groups
(tile_position (0,0)/(64,0)) so the hardware runs them concurrently,
doubling effective PE-array utilization for the scores phase (K=64 would
otherwise use half the array). PV keeps the ones-augmented V slab (M=65,
softmax denominator in row 64); PV for the previous pair is fed from a work
queue, a few matmuls per tj slot, using one [65, 1024] PSUM tile at a time.
NOTE: col-tiled concurrent matmul pairs (two PE writers into one PSUM bank)
fault the device — do not re-introduce them.

PSUM budget (8 banks): scores ring 2 x [128,1024] (4 banks) + aux ring
1 x [128,1024] (2, projection accumulators, emitted atomically per chunk)
+ PV accumulator 1 x [65,1024] (2).

Loop order everywhere keeps the col-group innermost so consecutive MMs share
the stationary operand (saves the PE weight-swap drain bubble).
"""

from collections import deque
from contextlib import ExitStack

import ml_dtypes
import numpy as np

import concourse.mybir as mybir
import concourse.tile as tile
from concourse import bacc
from concourse.bass_utils import run_bass_kernel_spmd

dt = mybir.dt
AF = mybir.ActivationFunctionType

B, N, D = 16, 784, 768
H, HD = 12, 64
F3 = 3 * D  # 2304
N_CORES = 8
BPC = B // N_CORES  # batches per core

T_CHUNKS = [(i * 128, min(128, N - i * 128)) for i in range((N + 127) // 128)]
NT = len(T_CHUNKS)  # 7
ND = D // 128  # 6
COLS_N = [(0, 512), (512, N - 512)]
COLS_D = [(0, 512), (512, D - 512)]

BF = dt.bfloat16


class _Slab:
    """View adapter: slab[di][:, c0:c1] -> big_tile[:, di*pitch+c0 : di*pitch+c1].

    Lets one [128, ND*pitch] SBUF tile (loaded by a few multi-block DMAs)
    stand in for a list of ND per-row-block tiles.
    """

    def __init__(self, tile, pitch):
        self.tile, self.pitch = tile, pitch

    def __getitem__(self, di):
        return _SlabRow(self.tile, di * self.pitch)


class _SlabRow:
    def __init__(self, tile, base):
        self.tile, self.base = tile, base

    def __getitem__(self, key):
        ps, cs = key
        return self.tile[ps, self.base + cs.start:self.base + cs.stop]


def _setup_consts(nc, P, aps, st):
    """Emit input DMAs in consumption-priority order across the two
    HW-DGE queues (sync, scalar) plus the gpsimd SW queue.

    Each dma_start costs ~600ns of sequencer issue time, so loads are
    coalesced: xt/wq/wo are single [128, ND*pitch] slabs filled by
    multi-row-block DMAs (3D access patterns), column-sliced to match
    the downstream consumption order (fi 0,6,1,7 -> V -> fi 2..5,8..11).
    Sub-tile dependency tracking lets each matmul wait only for the DMA
    piece covering the columns it reads.
    """
    xt0_t = P["xt"].tile([128, ND * N], BF, name="xt0", tag="xt")
    wq_t = P["wq"].tile([128, ND * F3], BF, name="wq", tag="wq")
    wo_t = P["wo"].tile([128, ND * D], BF, name="wo", tag="wo")
    xt0 = _Slab(xt0_t, N)
    wq16 = _Slab(wq_t, F3)
    wo16 = _Slab(wo_t, D)
    xsv = aps["xs"].rearrange("b (k p) n -> b p k n", k=ND)
    wqv = aps["wqkv"].rearrange("(k p) c -> p k c", k=ND)
    wov = aps["wo"].rearrange("(k p) c -> p k c", k=ND)
    xtv = xt0_t[:].rearrange("p (k n) -> p k n", k=ND)
    wqt = wq_t[:].rearrange("p (k c) -> p k c", k=ND)
    wot = wo_t[:].rearrange("p (k c) -> p k c", k=ND)

    def wq_dma(eng, c0, cw):
        eng.dma_start(wqt[:, :, c0:c0 + cw], wqv[:, :, c0:c0 + cw])

    # sync: x row-blocks 0-2, fi=0 cols, outer V col group
    nc.sync.dma_start(xtv[:, 0:3, :], xsv[0, :, 0:3, :])
    wq_dma(nc.sync, 0, 128)                              # fi=0
    wq_dma(nc.sync, 2 * D, COLS_D[0][1])                 # V cols 0:512
    # scalar: x row-blocks 3-5, fi=6, inner V col group, fi=7
    nc.scalar.dma_start(xtv[:, 3:6, :], xsv[0, :, 3:6, :])
    wq_dma(nc.scalar, 768, 128)                          # fi=6
    wq_dma(nc.scalar, 2 * D + COLS_D[1][0], COLS_D[1][1])  # V cols 512:768
    wq_dma(nc.scalar, 896, 128)                          # fi=7
    # gpsimd: biases + fi=1 early; late-consumed pieces after (b1's x,
    # Q/K cols fi 2-5/8-11 — consumed in c1 — and wo)
    bqc = P["konst"].tile([128, F3 // 128], dt.float32, name="bqc")
    nc.gpsimd.dma_start(bqc[:], aps["bqc"][:])
    wq_dma(nc.gpsimd, 128, 128)                          # fi=1
    bqv_r = P["konst"].tile([1, D], dt.float32, name="bqv_r")
    nc.gpsimd.dma_start(bqv_r[:], aps["bqv"][:])
    bqv_bc = P["konst"].tile([128, D], dt.float32, name="bqv_bc")
    nc.gpsimd.partition_broadcast(bqv_bc[:], bqv_r[0:1, :])
    bo_r = P["konst"].tile([1, D], dt.float32, name="bo_r")
    nc.gpsimd.dma_start(bo_r[:], aps["bo"][:])
    bo_bc = P["konst"].tile([128, D], dt.float32, name="bo_bc")
    nc.gpsimd.partition_broadcast(bo_bc[:], bo_r[0:1, :])
    st["xt0"] = xt0
    st["late_dmas"] = lambda: (
        wq_dma(nc.gpsimd, 256, 512),    # fi=2..5
        wq_dma(nc.gpsimd, 1024, 512),   # fi=8..11
        nc.gpsimd.dma_start(wot[:], wov[:]),
    )
    return dict(bqc=bqc, bqv_bc=bqv_bc, bo_bc=bo_bc, wq16=wq16, wo16=wo16)


def _gen_a(nc, P, C, aps, b, st):
    if b == 0:
        yield  # tiles already loaded by _setup_consts
        return
    xt_t = P["xt"].tile([128, ND * N], BF, name=f"xt{b}", tag="xt")
    xtv = xt_t[:].rearrange("p (k n) -> p k n", k=ND)
    xsv = aps["xs"].rearrange("b (k p) n -> b p k n", k=ND)
    nc.gpsimd.dma_start(xtv[:], xsv[b])
    st["late_dmas"]()
    yield
    st[f"xt{b}"] = _Slab(xt_t, N)


def _aux_ps(nc, P, st):
    """Projection PSUM allocator returning a slicing closure. Before
    attention PV starts (first ~8 tiles of batch 0), alternate between
    the aux pool and the then-idle PV pool (as two tiles matching its
    tag sizes) so drains double-buffer instead of serializing the PE."""
    n = st.get("auxn", 0)
    st["auxn"] = n + 1
    if n < 8 and n % 2 == 1:
        a = P["ps_ot"].tile([128, 512], dt.float32, name="pj_a", tag="ot0")
        b = P["ps_ot"].tile([128, 272], dt.float32, name="pj_b", tag="ot1")

        def sl(ps, c0, cw):
            if c0 >= 512:
                return b[ps, c0 - 512:c0 - 512 + cw]
            return a[ps, c0:c0 + cw]
        return sl
    t = P["ps_aux"].tile([128, 1024], dt.float32, name="pj_ps", tag="aux")

    def sl(ps, c0, cw):
        return t[ps, c0:c0 + cw]
    return sl


def _gen_b_qk(nc, P, C, b, st):
    """Q,K transposed layout: 12 tiles [128, 784]. di outer / cg inner."""
    xt16 = st[f"xt{b}"]
    qkt16 = [None] * 12
    st[f"qkt{b}"] = qkt16
    for fi in [0, 6, 1, 7, 2, 8, 3, 9, 4, 10, 5, 11]:
        qk_ps = _aux_ps(nc, P, st)
        for di in range(ND):
            for (c0, cw) in reversed(COLS_N):
                nc.tensor.matmul(
                    qk_ps(slice(None), c0, cw),
                    C["wq16"][di][:, fi * 128:(fi + 1) * 128],
                    xt16[di][:, c0:c0 + cw],
                    start=(di == 0), stop=(di == ND - 1))
            if di == 2:
                yield  # ~1us unit boundary mid-accumulation
        q = P["qkt"].tile([128, N], BF, name=f"qkt{b}_{fi}", tag="qkt")
        # drain bank B then A (matmul fill order is B, A) so the next aux
        # tile's first matmul only waits for the B-bank drain
        for (c0, cw) in reversed(COLS_N):
            nc.vector.tensor_scalar_add(q[:, c0:c0 + cw], qk_ps(slice(None), c0, cw),
                                        C["bqc"][0:128, fi:fi + 1])
        qkt16[fi] = q
        yield


def _gen_b_v(nc, P, C, b, st):
    """V slab [t, 12, 65]: 65th column is 1.0 (softmax denominator trick)."""
    xt16 = st[f"xt{b}"]
    vt16 = []
    st[f"vt{b}"] = vt16
    for (t0, p), ti in zip(T_CHUNKS, range(NT)):
        v_ps = _aux_ps(nc, P, st)
        for di in range(ND):
            for (c0, cw) in reversed(COLS_D):
                nc.tensor.matmul(
                    v_ps(slice(0, p), c0, cw),
                    xt16[di][:, t0:t0 + p],
                    C["wq16"][di][:, 2 * D + c0:2 * D + c0 + cw],
                    start=(di == 0), stop=(di == ND - 1))
            if di == 2:
                yield
        vt = P["vt"].tile([128, H, HD + 1], BF, name=f"vt{b}_{ti}", tag="vt")
        # drain per bank (features 512:768 = heads 8-11 fill first)
        for (h0, h1) in ((8, 12), (0, 8)):
            nc.vector.tensor_tensor(
                vt[0:p, h0:h1, 0:HD],
                v_ps(slice(0, p), h0 * HD, (h1 - h0) * HD).rearrange(
                    "p (h d) -> p h d", h=h1 - h0),
                C["bqv_bc"][0:p, h0 * HD:h1 * HD].rearrange("p (h d) -> p h d", h=h1 - h0),
                mybir.AluOpType.add)
        nc.vector.memset(vt[0:p, :, HD:HD + 1], 1.0)
        vt16.append(vt)
        yield


def _pv_thunks(nc, P, b, st, p):
    """Work queue for pair p's PV + normalize (two heads, sequential).

    Per head: two PSUM accumulators (one per col group, 1 bank each), 14
    accumulating MMs tj-outer / cg-inner (consecutive MMs share the vt
    stationary; ex consumption is tj-monotone), then drain + normalize
    into ot16[p] rows hh*64. Row 64 of the accumulators is the softmax
    denominator (ones column of the V slab).
    """
    vt16, ex = st[f"vt{b}"], st[f"ex{b}_{p}"]
    work = deque()
    holder = {}

    def _alloc(hh):
        def f():
            holder[0] = P["ps_ot"].tile([HD + 1, 512], dt.float32,
                                        name=f"pv0_{p}_{hh}", tag="ot0")
            holder[1] = P["ps_ot"].tile([HD + 1, COLS_N[1][1]], dt.float32,
                                        name=f"pv1_{p}_{hh}", tag="ot1")
        return f

    def _mms(hh, tjs):
        h = 2 * p + hh

        def f():
            for tj in tjs:
                (t0, pj) = T_CHUNKS[tj]
                for ci, (c0, cw) in reversed(list(enumerate(COLS_N))):
                    nc.tensor.matmul(
                        holder[ci][0:HD + 1, 0:cw],
                        vt16[tj][0:pj, h, 0:HD + 1],
                        ex[2 * tj + hh][0:pj, c0:c0 + cw],
                        start=(tj == 0), stop=(tj == NT - 1))
        return f

    fstate = {}

    def _finish_a(hh):
        """Drain holder to SBUF, reciprocal of the denominator row, and
        gpsimd broadcast. The multiply is deferred to _finish_b so head
        hh=1's vector work can overlap head hh=0's gpsimd broadcast."""
        def f():
            osb = P["osb"].tile([HD + 1, N], dt.float32, name="osb65", tag="osb")
            for ci, (c0, cw) in enumerate(COLS_N):
                nc.vector.tensor_copy(osb[0:HD + 1, c0:c0 + cw],
                                      holder[ci][0:HD + 1, 0:cw])
            srow = P["recp"].tile([1, N], dt.float32, name="srow", tag="srow")
            nc.vector.tensor_copy(srow[0:1, :], osb[HD:HD + 1, :])
            rec = P["recp"].tile([1, N], dt.float32, name="rec", tag="rec")
            nc.vector.reciprocal_approx_fast(rec[0:1, :], srow[0:1, :])
            brec = P["brec"].tile([HD, N], dt.float32, name="brec", tag="brec")
            nc.gpsimd.partition_broadcast(brec[0:HD, :], rec[0:1, :])
            fstate[hh] = (osb, brec)
        return f

    def _finish_b(hh):
        def f():
            if hh == 0:
                ot = P["ot"].tile([128, N], BF, name=f"ot{b}_{p}", tag="ot")
                st[f"ot{b}"][p] = ot
            ot = st[f"ot{b}"][p]
            osb, brec = fstate[hh]
            ro = hh * HD
            nc.vector.tensor_mul(ot[ro:ro + HD, :], osb[0:HD, :], brec[0:HD, :])
        return f

    for hh in range(2):
        work.append(_alloc(hh))
        work.append(_mms(hh, range(0, 2)))
        work.append(_mms(hh, range(2, 4)))
        work.append(_mms(hh, range(4, 6)))
        work.append(_mms(hh, range(6, NT)))
        work.append(_finish_a(hh))
    work.append(_finish_b(0))
    work.append(_finish_b(1))
    return work


def _pop(work, k):
    n = 0
    while work and n < k:
        work.popleft()()
        n += 1
    return n


def _gen_c(nc, P, C, b, st):
    """Attention for batch b. Yields once per (pair, tj)."""
    qkt16 = st[f"qkt{b}"]
    st[f"ot{b}"] = [None] * 6
    pv_work = deque()
    for p in range(6):
        ex = [P["ex"].tile([128, N], BF, name="ex", tag="ex") for _ in range(2 * NT)]
        st[f"ex{b}_{p}"] = ex
        if p >= 1:
            assert not pv_work
            pv_work = _pv_thunks(nc, P, b, st, p - 1)
        for (t0, pj), tj in zip(T_CHUNKS, range(NT)):
            # prev-pair PV (and the outer loop's filler, via yield) go in
            # front of the scores so the PE reaches the next score pair
            # after ACT has drained the previous one. At a pair's FIRST
            # iteration, defer 2 of the 3 pops until after the exps: the
            # measured ACT gaps sit at pair boundaries, where the full
            # pop batch + filler otherwise push the scores ~1us past the
            # PSUM-ring-free point.
            yield
            qt, kt = qkt16[p], qkt16[6 + p]
            sc_a = P["ps_sc"].tile([128, 1024], dt.float32, name="sc_a", tag="sc")
            sc_b = P["ps_sc"].tile([128, 1024], dt.float32, name="sc_b", tag="sc")
            # col-outer / head-inner: consecutive matmuls sit on disjoint
            # PE row quadrants (and different PSUM banks), so each pair
            # runs concurrently — issue order head-outer would serialize
            # the two heads' same-quadrant matmuls instead.
            for (c0, cw) in reversed(COLS_N):
                for hh, sc in ((0, sc_a), (1, sc_b)):
                    ro = hh * HD
                    nc.tensor.matmul(
                        sc[0:pj, c0:c0 + cw],
                        kt[ro:ro + HD, t0:t0 + pj],
                        qt[ro:ro + HD, c0:c0 + cw],
                        start=True, stop=True)
            for hh, sc in ((0, sc_a), (1, sc_b)):
                nc.scalar.activation(ex[2 * tj + hh][0:pj, :], sc[0:pj, 0:N],
                                     AF.Exp, scale=float(HD) ** -0.5)
            _pop(pv_work, 3)
        _pop(pv_work, 99)  # finish prev pair's PV before ex tiles rotate far
    # epilogue: last pair's PV, yielding so the caller can interleave the
    # output projection's early chunks
    pv_work = _pv_thunks(nc, P, b, st, 5)
    while pv_work:
        _pop(pv_work, 3)
        yield


def _gen_d(nc, P, C, aps, b, st):
    """Full output projection for batch b (used for b=0, scheduled during
    c1 where all of b0's ot tiles are ready). 14 yields: mms / drain per
    chunk, on the 1-buf aux pool."""
    ot16 = st[f"ot{b}"]
    tiles = [None] * NT

    def _mms(ti):
        (t0, p) = T_CHUNKS[ti]
        tiles[ti] = P["ps_aux"].tile([128, 1024], dt.float32, name="y_ps", tag="aux")
        y_ps = tiles[ti]
        for oi in range(ND):
            for (c0, cw) in reversed(COLS_D):
                nc.tensor.matmul(
                    y_ps[0:p, c0:c0 + cw],
                    ot16[oi][:, t0:t0 + p],
                    C["wo16"][oi][:, c0:c0 + cw],
                    start=(oi == 0), stop=(oi == ND - 1))

    def _drain(ti):
        (t0, p) = T_CHUNKS[ti]
        y16 = P["yout"].tile([128, D], BF, name="y16", tag="y16")
        for (c0, cw) in reversed(COLS_D):
            nc.vector.tensor_add(y16[0:p, c0:c0 + cw], tiles[ti][0:p, c0:c0 + cw],
                                 C["bo_bc"][0:p, c0:c0 + cw])
        # all y stores ride the sync queue: a dma_start costs ~667ns of
        # sequencer time, and the scalar sequencer also dispatches the
        # exp ACTIVATEs that pace the attention phase
        qeng = nc.sync
        qeng.dma_start(aps["ys"][b, t0:t0 + p, :], y16[0:p, :])

    for ti in range(NT):
        _mms(ti)
        yield
        _drain(ti)
        yield


def _gen_d1P(nc, P, C, aps, st):
    """Batch-1 projection, partial stage: accumulate oi 0:3 (head pairs
    0-2, ready mid-c1) into aux PSUM, drain (+bias) to a bf16 partial.
    7 yields."""
    ot16 = st["ot1"]
    parts = st.setdefault("d1parts", [None] * NT)
    for ti in range(NT):
        (t0, p) = T_CHUNKS[ti]
        y_ps = P["ps_aux"].tile([128, 1024], dt.float32, name="yp_ps", tag="aux")
        for oi in range(3):
            for (c0, cw) in reversed(COLS_D):
                nc.tensor.matmul(
                    y_ps[0:p, c0:c0 + cw],
                    ot16[oi][:, t0:t0 + p],
                    C["wo16"][oi][:, c0:c0 + cw],
                    start=(oi == 0), stop=(oi == 2))
        part = P["part"].tile([128, D], BF, name="ypart", tag="part")
        for (c0, cw) in reversed(COLS_D):
            nc.vector.tensor_add(part[0:p, c0:c0 + cw], y_ps[0:p, c0:c0 + cw],
                                 C["bo_bc"][0:p, c0:c0 + cw])
        parts[ti] = part
        yield


def _gen_d1F(nc, P, C, aps, st):
    """Batch-1 projection, final stage: oi 3:6 into sc PSUM (2 bufs) then
    y = psum + partial. Split per chunk into F1 (oi 3:5 — pairs 3,4) and
    F2 (oi 5 + drain) so F1 can be pulled before pair 5 finishes."""
    ot16 = st["ot1"]
    parts = st["d1parts"]
    tiles = [None] * NT

    def _f1(ti):
        (t0, p) = T_CHUNKS[ti]
        tiles[ti] = P["ps_sc"].tile([128, 1024], dt.float32, name="yf_ps", tag="sc")
        y_ps = tiles[ti]
        for oi in (3, 4):
            for (c0, cw) in reversed(COLS_D):
                nc.tensor.matmul(
                    y_ps[0:p, c0:c0 + cw],
                    ot16[oi][:, t0:t0 + p],
                    C["wo16"][oi][:, c0:c0 + cw],
                    start=(oi == 3), stop=False)

    def _f2(ti):
        (t0, p) = T_CHUNKS[ti]
        y_ps = tiles[ti]
        for (c0, cw) in reversed(COLS_D):
            nc.tensor.matmul(
                y_ps[0:p, c0:c0 + cw],
                ot16[5][:, t0:t0 + p],
                C["wo16"][5][:, c0:c0 + cw],
                start=False, stop=True)
        y16 = P["yout"].tile([128, D], BF, name="y16", tag="y16")
        for (c0, cw) in reversed(COLS_D):
            nc.vector.tensor_add(y16[0:p, c0:c0 + cw], y_ps[0:p, c0:c0 + cw],
                                 parts[ti][0:p, c0:c0 + cw])
        # all y stores ride the sync queue: a dma_start costs ~667ns of
        # sequencer time, and the scalar sequencer also dispatches the
        # exp ACTIVATEs that pace the attention phase
        qeng = nc.sync
        qeng.dma_start(aps["ys"][1, t0:t0 + p, :], y16[0:p, :])

    _f1(0)
    yield
    _f1(1)
    yield
    _f2(0)
    yield
    for ti in range(2, NT):
        _f1(ti)
        yield
        _f2(ti - 1)
        yield
    _f2(NT - 1)


def _exhaust(g):
    for _ in g:
        pass


def _pull(g, k):
    n = 0
    for _ in range(k):
        try:
            next(g)
        except StopIteration:
            break
        n += 1
    return n


POOL_SPECS = [
    ("konst", 1, "SBUF"), ("wq", 1, "SBUF"), ("wo", 1, "SBUF"),
    ("xt", 2, "SBUF"),
    ("qkt", 20, "SBUF"), ("vt", 2 * NT, "SBUF"), ("ex", 18, "SBUF"),
    ("ot", 12, "SBUF"), ("osb", 3, "SBUF"), ("recp", 2, "SBUF"),
    ("brec", 2, "SBUF"),
    ("yout", 2, "SBUF"), ("part", 8, "SBUF"),
    ("ps_sc", 2, "PSUM"), ("ps_aux", 1, "PSUM"), ("ps_ot", 1, "PSUM"),
]


def build():
    nc = bacc.Bacc("TRN2", target_bir_lowering=False, debug=False)

    aps = {
        "xs": nc.dram_tensor("xs", [BPC, D, N], BF, kind="ExternalInput").ap(),
        "wqkv": nc.dram_tensor("wqkv", [D, F3], BF, kind="ExternalInput").ap(),
        "bqc": nc.dram_tensor("bqc", [128, F3 // 128], dt.float32, kind="ExternalInput").ap(),
        "bqv": nc.dram_tensor("bqv", [1, D], dt.float32, kind="ExternalInput").ap(),
        "wo": nc.dram_tensor("wo", [D, D], BF, kind="ExternalInput").ap(),
        "bo": nc.dram_tensor("bo", [1, D], dt.float32, kind="ExternalInput").ap(),
        "ys": nc.dram_tensor("ys", [BPC, N, D], BF, kind="ExternalOutput").ap(),
    }

    with ExitStack() as ctx:
        tc = ctx.enter_context(tile.TileContext(nc))
        P = {}
        for pname, bufs, space in POOL_SPECS:
            P[pname] = ctx.enter_context(
                tc.tile_pool(name=pname, bufs=bufs, space=space))

        st = {}
        C = _setup_consts(nc, P, aps, st)
        import itertools
        def _interleave(gq, gv):
            # qk pair chunks (2 per attention pair) woven with V chunks;
            # fi {0,6,1,7} first to match the DMA piece priority order.
            # Each tile is two ~1us yield-units now, so double each char.
            order = "".join(c * 2 for c in "qqqqvvvvvvvqqqqqqqq")
            for ch in order:
                g = gq if ch == "q" else gv
                try:
                    yield next(g)
                except StopIteration:
                    pass

        f0 = itertools.chain(
            _gen_a(nc, P, C, aps, 0, st),
            _interleave(_gen_b_qk(nc, P, C, 0, st), _gen_b_v(nc, P, C, 0, st)))
        f1 = itertools.chain(
            _gen_a(nc, P, C, aps, 1, st),
            _interleave(_gen_b_qk(nc, P, C, 1, st), _gen_b_v(nc, P, C, 1, st)))
        _pull(f0, 7)  # xt-b0 record + qk fi0, fi6, fi1 (2 units each)
        _pull(f1, 1)  # xt-b1 DMA (queued behind the startup pieces)

        # c0 (batch-0 attention): b0's prep as ~1us units — dense while
        # the V slab is needed (2/yield through i=7), then one unit per
        # yield so filler lumps between the PV pops and the scores stay
        # small and the exps are never pushed late. During c0's
        # epilogue, also drive c1's first yields so batch-1's pair-0
        # scores overlap c0's tail.
        c0 = _gen_c(nc, P, C, 0, st)
        c1 = _gen_c(nc, P, C, 1, st)
        i = 0
        for _ in c0:
            if i >= 42:
                _pull(c1, 1)                 # c1 scores ramp (yields 0-4)
                _pull(f1, 1)                 # v4u2, v5, v6 of b1
            elif i < 8:
                _pull(f0, 2)                 # fi7 + v0-v6 (vt6 by i=7)
            elif i <= 23:
                _pull(f0, 1)                 # fi 2,8,3,9,4,10,5,11 units
            elif i >= 25:
                _pull(f1, 1)                 # b1: fi0,6,1,7 + v0-v4u1
            i += 1
        # c1 (batch-1 attention, yields 5-46): b1's last qk units on odd
        # 5-35, d0 on even 6-32, d1 partials on even 34-46, F1 on 41/43.
        d0 = _gen_d(nc, P, C, aps, 0, st)
        d1p = _gen_d1P(nc, P, C, aps, st)
        d1f = _gen_d1F(nc, P, C, aps, st)
        j = 5
        for _ in c1:
            if j % 2 == 1 and j <= 35:
                _pull(f1, 1)                 # fi 2..11 remaining units
            elif j % 2 == 0 and 6 <= j <= 32:
                _pull(d0, 1)
            elif j % 2 == 0 and j >= 34:
                _pull(d1p, 1)
            elif j in (41, 43):
                _pull(d1f, 1)                # F1 chunks 0,1 (pairs 3,4)
            j += 1
        _exhaust(f0)
        _exhaust(f1)
        _exhaust(d0)
        _exhaust(d1p)
        _exhaust(d1f)

    nc.compile()
    return nc


_NC_CACHE = {}


def _get_nc():
    if "nc" not in _NC_CACHE:
        _NC_CACHE["nc"] = build()
    return _NC_CACHE["nc"]


def make_in_maps(x, Wqkv, bqkv, Wo, bo):
    bf = ml_dtypes.bfloat16
    x = np.asarray(x, dtype=np.float32)
    Wqkv16 = np.ascontiguousarray(np.asarray(Wqkv, np.float32).astype(bf))
    bqkv = np.asarray(bqkv, dtype=np.float32)
    Wo16 = np.ascontiguousarray(np.asarray(Wo, np.float32).astype(bf))
    bo = np.asarray(bo, dtype=np.float32)
    bqc = np.ascontiguousarray(bqkv.reshape(F3 // 128, 128).T)
    bqv = np.ascontiguousarray(bqkv[2 * D:].reshape(1, D))
    bo_r = np.ascontiguousarray(bo.reshape(1, D))
    x16 = np.ascontiguousarray(x.astype(bf).transpose(0, 2, 1))
    in_maps = []
    for c in range(N_CORES):
        in_maps.append({
            "xs": np.ascontiguousarray(x16[c * BPC:(c + 1) * BPC]),
            "wqkv": Wqkv16,
            "bqc": bqc,
            "bqv": bqv,
            "wo": Wo16,
            "bo": bo_r,
        })
    return in_maps


def run(x, Wqkv, bqkv, Wo, bo, trace=False, **kw):
    nc = _get_nc()
    in_maps = make_in_maps(x, Wqkv, bqkv, Wo, bo)
    res = run_bass_kernel_spmd(nc, in_maps, list(range(N_CORES)), trace=trace, **kw)
    out = np.concatenate(
        [np.asarray(res.results[c]["ys"]).astype(np.float32) for c in range(N_CORES)],
        axis=0)
    return out, res


def kernel(x, Wqkv, bqkv, Wo, bo):
    out, _ = run(x, Wqkv, bqkv, Wo, bo)
    return out



# revision 95
# speedup vs baseline: 1.0063x; 1.0003x over previous
"""Multi-head self-attention (B=16, N=784, D=768, H=12) on 8 trn2 cores.

Sharding: pure data-parallel over batch (2 batches per core, no collectives).
All matmuls bf16 with fp32 PSUM accumulation.

v2 over baseline: score matmuls for a head pair (which share one qkt tile —
head 2p at partitions 0:64, head 2p+1 at 64:128) are emitted back-to-back
into two separate PSUM tiles. They use disjoint PE row groups
(tile_position (0,0)/(64,0)) so the hardware runs them concurrently,
doubling effective PE-array utilization for the scores phase (K=64 would
otherwise use half the array). PV keeps the ones-augmented V slab (M=65,
softmax denominator in row 64); PV for the previous pair is fed from a work
queue, a few matmuls per tj slot, using one [65, 1024] PSUM tile at a time.
NOTE: col-tiled concurrent matmul pairs (two PE writers into one PSUM bank)
fault the device — do not re-introduce them.

PSUM budget (8 banks): scores ring 2 x [128,1024] (4 banks) + aux ring
1 x [128,1024] (2, projection accumulators, emitted atomically per chunk)
+ PV accumulator 1 x [65,1024] (2).

Loop order everywhere keeps the col-group innermost so consecutive MMs share
the stationary operand (saves the PE weight-swap drain bubble).
"""

from collections import deque
from contextlib import ExitStack

import ml_dtypes
import numpy as np

import concourse.mybir as mybir
import concourse.tile as tile
from concourse import bacc
from concourse.bass_utils import run_bass_kernel_spmd

dt = mybir.dt
AF = mybir.ActivationFunctionType

B, N, D = 16, 784, 768
H, HD = 12, 64
F3 = 3 * D  # 2304
N_CORES = 8
BPC = B // N_CORES  # batches per core

T_CHUNKS = [(i * 128, min(128, N - i * 128)) for i in range((N + 127) // 128)]
NT = len(T_CHUNKS)  # 7
ND = D // 128  # 6
COLS_N = [(0, 512), (512, N - 512)]
COLS_D = [(0, 512), (512, D - 512)]

BF = dt.bfloat16


class _Slab:
    """View adapter: slab[di][:, c0:c1] -> big_tile[:, di*pitch+c0 : di*pitch+c1].

    Lets one [128, ND*pitch] SBUF tile (loaded by a few multi-block DMAs)
    stand in for a list of ND per-row-block tiles.
    """

    def __init__(self, tile, pitch):
        self.tile, self.pitch = tile, pitch

    def __getitem__(self, di):
        return _SlabRow(self.tile, di * self.pitch)


class _SlabRow:
    def __init__(self, tile, base):
        self.tile, self.base = tile, base

    def __getitem__(self, key):
        ps, cs = key
        return self.tile[ps, self.base + cs.start:self.base + cs.stop]


def _setup_consts(nc, P, aps, st):
    """Emit input DMAs in consumption-priority order across the two
    HW-DGE queues (sync, scalar) plus the gpsimd SW queue.

    Each dma_start costs ~600ns of sequencer issue time, so loads are
    coalesced: xt/wq/wo are single [128, ND*pitch] slabs filled by
    multi-row-block DMAs (3D access patterns), column-sliced to match
    the downstream consumption order (fi 0,6,1,7 -> V -> fi 2..5,8..11).
    Sub-tile dependency tracking lets each matmul wait only for the DMA
    piece covering the columns it reads.
    """
    xt0_t = P["xt"].tile([128, ND * N], BF, name="xt0", tag="xt")
    wq_t = P["wq"].tile([128, ND * F3], BF, name="wq", tag="wq")
    wo_t = P["wo"].tile([128, ND * D], BF, name="wo", tag="wo")
    xt0 = _Slab(xt0_t, N)
    wq16 = _Slab(wq_t, F3)
    wo16 = _Slab(wo_t, D)
    xsv = aps["xs"].rearrange("b (k p) n -> b p k n", k=ND)
    wqv = aps["wqkv"].rearrange("(k p) c -> p k c", k=ND)
    wov = aps["wo"].rearrange("(k p) c -> p k c", k=ND)
    xtv = xt0_t[:].rearrange("p (k n) -> p k n", k=ND)
    wqt = wq_t[:].rearrange("p (k c) -> p k c", k=ND)
    wot = wo_t[:].rearrange("p (k c) -> p k c", k=ND)

    def wq_dma(eng, c0, cw):
        eng.dma_start(wqt[:, :, c0:c0 + cw], wqv[:, :, c0:c0 + cw])

    # sync: x row-blocks 0-2, fi=0 cols, outer V col group
    nc.sync.dma_start(xtv[:, 0:3, :], xsv[0, :, 0:3, :])
    wq_dma(nc.sync, 0, 128)                              # fi=0
    wq_dma(nc.sync, 2 * D, COLS_D[0][1])                 # V cols 0:512
    # scalar: x row-blocks 3-5, fi=6, inner V col group, fi=7
    nc.scalar.dma_start(xtv[:, 3:6, :], xsv[0, :, 3:6, :])
    wq_dma(nc.scalar, 768, 128)                          # fi=6
    wq_dma(nc.scalar, 2 * D + COLS_D[1][0], COLS_D[1][1])  # V cols 512:768
    wq_dma(nc.scalar, 896, 128)                          # fi=7
    # gpsimd: biases + fi=1 early; late-consumed pieces after (b1's x,
    # Q/K cols fi 2-5/8-11 — consumed in c1 — and wo)
    bqc = P["konst"].tile([128, F3 // 128], dt.float32, name="bqc")
    nc.gpsimd.dma_start(bqc[:], aps["bqc"][:])
    wq_dma(nc.gpsimd, 128, 128)                          # fi=1
    bqv_r = P["konst"].tile([1, D], dt.float32, name="bqv_r")
    nc.gpsimd.dma_start(bqv_r[:], aps["bqv"][:])
    bqv_bc = P["konst"].tile([128, D], dt.float32, name="bqv_bc")
    nc.gpsimd.partition_broadcast(bqv_bc[:], bqv_r[0:1, :])
    bo_r = P["konst"].tile([1, D], dt.float32, name="bo_r")
    nc.gpsimd.dma_start(bo_r[:], aps["bo"][:])
    bo_bc = P["konst"].tile([128, D], dt.float32, name="bo_bc")
    nc.gpsimd.partition_broadcast(bo_bc[:], bo_r[0:1, :])
    st["xt0"] = xt0
    st["late_dmas"] = lambda: (
        wq_dma(nc.gpsimd, 256, 512),    # fi=2..5
        wq_dma(nc.gpsimd, 1024, 512),   # fi=8..11
        nc.gpsimd.dma_start(wot[:], wov[:]),
    )
    return dict(bqc=bqc, bqv_bc=bqv_bc, bo_bc=bo_bc, wq16=wq16, wo16=wo16)


def _gen_a(nc, P, C, aps, b, st):
    if b == 0:
        yield  # tiles already loaded by _setup_consts
        return
    xt_t = P["xt"].tile([128, ND * N], BF, name=f"xt{b}", tag="xt")
    xtv = xt_t[:].rearrange("p (k n) -> p k n", k=ND)
    xsv = aps["xs"].rearrange("b (k p) n -> b p k n", k=ND)
    nc.gpsimd.dma_start(xtv[:], xsv[b])
    st["late_dmas"]()
    yield
    st[f"xt{b}"] = _Slab(xt_t, N)


def _aux_ps(nc, P, st):
    """Projection PSUM allocator returning a slicing closure. Before
    attention PV starts (first ~8 tiles of batch 0), alternate between
    the aux pool and the then-idle PV pool (as two tiles matching its
    tag sizes) so drains double-buffer instead of serializing the PE."""
    n = st.get("auxn", 0)
    st["auxn"] = n + 1
    if n < 8 and n % 2 == 1:
        a = P["ps_ot"].tile([128, 512], dt.float32, name="pj_a", tag="ot0")
        b = P["ps_ot"].tile([128, 272], dt.float32, name="pj_b", tag="ot1")

        def sl(ps, c0, cw):
            if c0 >= 512:
                return b[ps, c0 - 512:c0 - 512 + cw]
            return a[ps, c0:c0 + cw]
        return sl
    t = P["ps_aux"].tile([128, 1024], dt.float32, name="pj_ps", tag="aux")

    def sl(ps, c0, cw):
        return t[ps, c0:c0 + cw]
    return sl


def _gen_b_qk(nc, P, C, b, st):
    """Q,K transposed layout: 12 tiles [128, 784]. di outer / cg inner."""
    xt16 = st[f"xt{b}"]
    qkt16 = [None] * 12
    st[f"qkt{b}"] = qkt16
    for fi in [0, 6, 1, 7, 2, 8, 3, 9, 4, 10, 5, 11]:
        qk_ps = _aux_ps(nc, P, st)
        for di in range(ND):
            for (c0, cw) in reversed(COLS_N):
                nc.tensor.matmul(
                    qk_ps(slice(None), c0, cw),
                    C["wq16"][di][:, fi * 128:(fi + 1) * 128],
                    xt16[di][:, c0:c0 + cw],
                    start=(di == 0), stop=(di == ND - 1))
            if di == 2:
                yield  # ~1us unit boundary mid-accumulation
        q = P["qkt"].tile([128, N], BF, name=f"qkt{b}_{fi}", tag="qkt")
        # drain bank B then A (matmul fill order is B, A) so the next aux
        # tile's first matmul only waits for the B-bank drain
        for (c0, cw) in reversed(COLS_N):
            nc.vector.tensor_scalar_add(q[:, c0:c0 + cw], qk_ps(slice(None), c0, cw),
                                        C["bqc"][0:128, fi:fi + 1])
        qkt16[fi] = q
        yield


def _gen_b_v(nc, P, C, b, st):
    """V slab [t, 12, 65]: 65th column is 1.0 (softmax denominator trick)."""
    xt16 = st[f"xt{b}"]
    vt16 = []
    st[f"vt{b}"] = vt16
    for (t0, p), ti in zip(T_CHUNKS, range(NT)):
        v_ps = _aux_ps(nc, P, st)
        for di in range(ND):
            for (c0, cw) in reversed(COLS_D):
                nc.tensor.matmul(
                    v_ps(slice(0, p), c0, cw),
                    xt16[di][:, t0:t0 + p],
                    C["wq16"][di][:, 2 * D + c0:2 * D + c0 + cw],
                    start=(di == 0), stop=(di == ND - 1))
            if di == 2:
                yield
        vt = P["vt"].tile([128, H, HD + 1], BF, name=f"vt{b}_{ti}", tag="vt")
        # drain per bank (features 512:768 = heads 8-11 fill first)
        for (h0, h1) in ((8, 12), (0, 8)):
            nc.vector.tensor_tensor(
                vt[0:p, h0:h1, 0:HD],
                v_ps(slice(0, p), h0 * HD, (h1 - h0) * HD).rearrange(
                    "p (h d) -> p h d", h=h1 - h0),
                C["bqv_bc"][0:p, h0 * HD:h1 * HD].rearrange("p (h d) -> p h d", h=h1 - h0),
                mybir.AluOpType.add)
        nc.vector.memset(vt[0:p, :, HD:HD + 1], 1.0)
        vt16.append(vt)
        yield


def _pv_thunks(nc, P, b, st, p):
    """Work queue for pair p's PV + normalize (two heads, sequential).

    Per head: two PSUM accumulators (one per col group, 1 bank each), 14
    accumulating MMs tj-outer / cg-inner (consecutive MMs share the vt
    stationary; ex consumption is tj-monotone), then drain + normalize
    into ot16[p] rows hh*64. Row 64 of the accumulators is the softmax
    denominator (ones column of the V slab).
    """
    vt16, ex = st[f"vt{b}"], st[f"ex{b}_{p}"]
    work = deque()
    holder = {}

    def _alloc(hh):
        def f():
            holder[0] = P["ps_ot"].tile([HD + 1, 512], dt.float32,
                                        name=f"pv0_{p}_{hh}", tag="ot0")
            holder[1] = P["ps_ot"].tile([HD + 1, COLS_N[1][1]], dt.float32,
                                        name=f"pv1_{p}_{hh}", tag="ot1")
        return f

    def _mms(hh, tjs):
        h = 2 * p + hh

        def f():
            for tj in tjs:
                (t0, pj) = T_CHUNKS[tj]
                for ci, (c0, cw) in reversed(list(enumerate(COLS_N))):
                    nc.tensor.matmul(
                        holder[ci][0:HD + 1, 0:cw],
                        vt16[tj][0:pj, h, 0:HD + 1],
                        ex[2 * tj + hh][0:pj, c0:c0 + cw],
                        start=(tj == 0), stop=(tj == NT - 1))
        return f

    fstate = {}

    def _finish_a(hh):
        """Drain holder to SBUF, reciprocal of the denominator row, and
        gpsimd broadcast. The multiply is deferred to _finish_b so head
        hh=1's vector work can overlap head hh=0's gpsimd broadcast."""
        def f():
            osb = P["osb"].tile([HD + 1, N], dt.float32, name="osb65", tag="osb")
            for ci, (c0, cw) in enumerate(COLS_N):
                nc.vector.tensor_copy(osb[0:HD + 1, c0:c0 + cw],
                                      holder[ci][0:HD + 1, 0:cw])
            srow = P["recp"].tile([1, N], dt.float32, name="srow", tag="srow")
            nc.vector.tensor_copy(srow[0:1, :], osb[HD:HD + 1, :])
            rec = P["recp"].tile([1, N], dt.float32, name="rec", tag="rec")
            nc.vector.reciprocal_approx_fast(rec[0:1, :], srow[0:1, :])
            brec = P["brec"].tile([HD, N], dt.float32, name="brec", tag="brec")
            nc.gpsimd.partition_broadcast(brec[0:HD, :], rec[0:1, :])
            fstate[hh] = (osb, brec)
        return f

    def _finish_b(hh):
        def f():
            if hh == 0:
                ot = P["ot"].tile([128, N], BF, name=f"ot{b}_{p}", tag="ot")
                st[f"ot{b}"][p] = ot
            ot = st[f"ot{b}"][p]
            osb, brec = fstate[hh]
            ro = hh * HD
            nc.vector.tensor_mul(ot[ro:ro + HD, :], osb[0:HD, :], brec[0:HD, :])
        return f

    for hh in range(2):
        work.append(_alloc(hh))
        work.append(_mms(hh, range(0, 2)))
        work.append(_mms(hh, range(2, 4)))
        work.append(_mms(hh, range(4, 6)))
        work.append(_mms(hh, range(6, NT)))
        work.append(_finish_a(hh))
    work.append(_finish_b(0))
    work.append(_finish_b(1))
    return work


def _pop(work, k):
    n = 0
    while work and n < k:
        work.popleft()()
        n += 1
    return n


def _gen_c(nc, P, C, b, st):
    """Attention for batch b. Yields once per (pair, tj)."""
    qkt16 = st[f"qkt{b}"]
    st[f"ot{b}"] = [None] * 6
    pv_work = deque()
    for p in range(6):
        ex = [P["ex"].tile([128, N], BF, name="ex", tag="ex") for _ in range(2 * NT)]
        st[f"ex{b}_{p}"] = ex
        if p >= 1:
            assert not pv_work
            pv_work = _pv_thunks(nc, P, b, st, p - 1)
        for (t0, pj), tj in zip(T_CHUNKS, range(NT)):
            # prev-pair PV (and the outer loop's filler, via yield) go in
            # front of the scores so the PE reaches the next score pair
            # after ACT has drained the previous one. At a pair's FIRST
            # iteration, defer 2 of the 3 pops until after the exps: the
            # measured ACT gaps sit at pair boundaries, where the full
            # pop batch + filler otherwise push the scores ~1us past the
            # PSUM-ring-free point.
            yield
            qt, kt = qkt16[p], qkt16[6 + p]
            sc_a = P["ps_sc"].tile([128, 1024], dt.float32, name="sc_a", tag="sc")
            sc_b = P["ps_sc"].tile([128, 1024], dt.float32, name="sc_b", tag="sc")
            # col-outer / head-inner: consecutive matmuls sit on disjoint
            # PE row quadrants (and different PSUM banks), so each pair
            # runs concurrently — issue order head-outer would serialize
            # the two heads' same-quadrant matmuls instead.
            for (c0, cw) in reversed(COLS_N):
                for hh, sc in ((0, sc_a), (1, sc_b)):
                    ro = hh * HD
                    nc.tensor.matmul(
                        sc[0:pj, c0:c0 + cw],
                        kt[ro:ro + HD, t0:t0 + pj],
                        qt[ro:ro + HD, c0:c0 + cw],
                        start=True, stop=True)
            for hh, sc in ((0, sc_a), (1, sc_b)):
                nc.scalar.activation(ex[2 * tj + hh][0:pj, :], sc[0:pj, 0:N],
                                     AF.Exp, scale=float(HD) ** -0.5)
            _pop(pv_work, 3)
            _pop(st["xw"], 1)  # extra post-scores work (d0 units in c1)
        _pop(pv_work, 99)  # finish prev pair's PV before ex tiles rotate far
    # epilogue: last pair's PV, yielding so the caller can interleave the
    # output projection's early chunks
    pv_work = _pv_thunks(nc, P, b, st, 5)
    while pv_work:
        _pop(pv_work, 3)
        yield


def _d0_items(nc, P, C, aps, b, st):
    """Batch-0 output projection as ~1.1us work items for the shared
    post-scores queue (popped inside c1's iterations, after the exps,
    where they cannot delay ACT's inputs)."""
    ot16 = st[f"ot{b}"]
    tiles = [None] * NT

    def _mms(ti, ois, alloc):
        (t0, p) = T_CHUNKS[ti]
        if alloc:
            tiles[ti] = P["ps_aux"].tile([128, 1024], dt.float32,
                                         name="y_ps", tag="aux")
        y_ps = tiles[ti]
        for oi in ois:
            for (c0, cw) in reversed(COLS_D):
                nc.tensor.matmul(
                    y_ps[0:p, c0:c0 + cw],
                    ot16[oi][:, t0:t0 + p],
                    C["wo16"][oi][:, c0:c0 + cw],
                    start=(oi == 0), stop=(oi == ND - 1))

    def _drain(ti):
        (t0, p) = T_CHUNKS[ti]
        y16 = P["yout"].tile([128, D], BF, name="y16", tag="y16")
        for (c0, cw) in reversed(COLS_D):
            nc.vector.tensor_add(y16[0:p, c0:c0 + cw], tiles[ti][0:p, c0:c0 + cw],
                                 C["bo_bc"][0:p, c0:c0 + cw])
        # all y stores ride the sync queue: a dma_start costs ~667ns of
        # sequencer time, and the scalar sequencer also dispatches the
        # exp ACTIVATEs that pace the attention phase
        qeng = nc.sync
        qeng.dma_start(aps["ys"][b, t0:t0 + p, :], y16[0:p, :])

    items = []
    for ti in range(NT):
        items.append(lambda ti=ti: _mms(ti, range(0, 3), True))
        items.append(lambda ti=ti: _mms(ti, range(3, ND), False))
        items.append(lambda ti=ti: _drain(ti))
    return items


def _gen_d1P(nc, P, C, aps, st):
    """Batch-1 projection, partial stage: accumulate oi 0:3 (head pairs
    0-2, ready mid-c1) into aux PSUM, drain (+bias) to a bf16 partial.
    7 yields."""
    ot16 = st["ot1"]
    parts = st.setdefault("d1parts", [None] * NT)
    for ti in range(NT):
        (t0, p) = T_CHUNKS[ti]
        y_ps = P["ps_aux"].tile([128, 1024], dt.float32, name="yp_ps", tag="aux")
        for oi in range(3):
            for (c0, cw) in reversed(COLS_D):
                nc.tensor.matmul(
                    y_ps[0:p, c0:c0 + cw],
                    ot16[oi][:, t0:t0 + p],
                    C["wo16"][oi][:, c0:c0 + cw],
                    start=(oi == 0), stop=(oi == 2))
        part = P["part"].tile([128, D], BF, name="ypart", tag="part")
        for (c0, cw) in reversed(COLS_D):
            nc.vector.tensor_add(part[0:p, c0:c0 + cw], y_ps[0:p, c0:c0 + cw],
                                 C["bo_bc"][0:p, c0:c0 + cw])
        parts[ti] = part
        yield


def _gen_d1F(nc, P, C, aps, st):
    """Batch-1 projection, final stage: oi 3:6 into sc PSUM (2 bufs) then
    y = psum + partial. Split per chunk into F1 (oi 3:5 — pairs 3,4) and
    F2 (oi 5 + drain) so F1 can be pulled before pair 5 finishes."""
    ot16 = st["ot1"]
    parts = st["d1parts"]
    tiles = [None] * NT

    def _f1(ti):
        (t0, p) = T_CHUNKS[ti]
        tiles[ti] = P["ps_sc"].tile([128, 1024], dt.float32, name="yf_ps", tag="sc")
        y_ps = tiles[ti]
        for oi in (3, 4):
            for (c0, cw) in reversed(COLS_D):
                nc.tensor.matmul(
                    y_ps[0:p, c0:c0 + cw],
                    ot16[oi][:, t0:t0 + p],
                    C["wo16"][oi][:, c0:c0 + cw],
                    start=(oi == 3), stop=False)

    def _f2(ti):
        (t0, p) = T_CHUNKS[ti]
        y_ps = tiles[ti]
        for (c0, cw) in reversed(COLS_D):
            nc.tensor.matmul(
                y_ps[0:p, c0:c0 + cw],
                ot16[5][:, t0:t0 + p],
                C["wo16"][5][:, c0:c0 + cw],
                start=False, stop=True)
        y16 = P["yout"].tile([128, D], BF, name="y16", tag="y16")
        for (c0, cw) in reversed(COLS_D):
            nc.vector.tensor_add(y16[0:p, c0:c0 + cw], y_ps[0:p, c0:c0 + cw],
                                 parts[ti][0:p, c0:c0 + cw])
        # all y stores ride the sync queue: a dma_start costs ~667ns of
        # sequencer time, and the scalar sequencer also dispatches the
        # exp ACTIVATEs that pace the attention phase
        qeng = nc.sync
        qeng.dma_start(aps["ys"][1, t0:t0 + p, :], y16[0:p, :])

    _f1(0)
    yield
    _f1(1)
    yield
    _f2(0)
    yield
    for ti in range(2, NT):
        _f1(ti)
        yield
        _f2(ti - 1)
        yield
    _f2(NT - 1)


def _exhaust(g):
    for _ in g:
        pass


def _pull(g, k):
    n = 0
    for _ in range(k):
        try:
            next(g)
        except StopIteration:
            break
        n += 1
    return n


POOL_SPECS = [
    ("konst", 1, "SBUF"), ("wq", 1, "SBUF"), ("wo", 1, "SBUF"),
    ("xt", 2, "SBUF"),
    ("qkt", 20, "SBUF"), ("vt", 2 * NT, "SBUF"), ("ex", 18, "SBUF"),
    ("ot", 12, "SBUF"), ("osb", 3, "SBUF"), ("recp", 2, "SBUF"),
    ("brec", 2, "SBUF"),
    ("yout", 2, "SBUF"), ("part", 8, "SBUF"),
    ("ps_sc", 2, "PSUM"), ("ps_aux", 1, "PSUM"), ("ps_ot", 1, "PSUM"),
]


def build():
    nc = bacc.Bacc("TRN2", target_bir_lowering=False, debug=False)

    aps = {
        "xs": nc.dram_tensor("xs", [BPC, D, N], BF, kind="ExternalInput").ap(),
        "wqkv": nc.dram_tensor("wqkv", [D, F3], BF, kind="ExternalInput").ap(),
        "bqc": nc.dram_tensor("bqc", [128, F3 // 128], dt.float32, kind="ExternalInput").ap(),
        "bqv": nc.dram_tensor("bqv", [1, D], dt.float32, kind="ExternalInput").ap(),
        "wo": nc.dram_tensor("wo", [D, D], BF, kind="ExternalInput").ap(),
        "bo": nc.dram_tensor("bo", [1, D], dt.float32, kind="ExternalInput").ap(),
        "ys": nc.dram_tensor("ys", [BPC, N, D], BF, kind="ExternalOutput").ap(),
    }

    with ExitStack() as ctx:
        tc = ctx.enter_context(tile.TileContext(nc))
        P = {}
        for pname, bufs, space in POOL_SPECS:
            P[pname] = ctx.enter_context(
                tc.tile_pool(name=pname, bufs=bufs, space=space))

        st = {}
        C = _setup_consts(nc, P, aps, st)
        import itertools
        def _interleave(gq, gv):
            # qk pair chunks (2 per attention pair) woven with V chunks;
            # fi {0,6,1,7} first to match the DMA piece priority order.
            # Each tile is two ~1us yield-units now, so double each char.
            order = "".join(c * 2 for c in "qqqqvvvvvvvqqqqqqqq")
            for ch in order:
                g = gq if ch == "q" else gv
                try:
                    yield next(g)
                except StopIteration:
                    pass

        f0 = itertools.chain(
            _gen_a(nc, P, C, aps, 0, st),
            _interleave(_gen_b_qk(nc, P, C, 0, st), _gen_b_v(nc, P, C, 0, st)))
        f1 = itertools.chain(
            _gen_a(nc, P, C, aps, 1, st),
            _interleave(_gen_b_qk(nc, P, C, 1, st), _gen_b_v(nc, P, C, 1, st)))
        st["xw"] = deque()
        _pull(f0, 7)  # xt-b0 record + qk fi0, fi6, fi1 (2 units each)
        _pull(f1, 1)  # xt-b1 DMA (queued behind the startup pieces)

        # c0 (batch-0 attention): b0's prep as ~1us units — dense while
        # the V slab is needed (2/yield through i=7), then one unit per
        # yield so filler lumps between the PV pops and the scores stay
        # small and the exps are never pushed late. During c0's
        # epilogue, also drive c1's first yields so batch-1's pair-0
        # scores overlap c0's tail.
        c0 = _gen_c(nc, P, C, 0, st)
        c1 = _gen_c(nc, P, C, 1, st)
        i = 0
        for _ in c0:
            if i >= 42:
                _pull(c1, 1)                 # c1 scores ramp (yields 0-4)
                _pull(f1, 1)                 # v4u2, v5, v6 of b1
            elif i < 8:
                _pull(f0, 2)                 # fi7 + v0-v6 (vt6 by i=7)
            elif i <= 23:
                _pull(f0, 1)                 # fi 2,8,3,9,4,10,5,11 units
            elif i >= 25:
                _pull(f1, 1)                 # b1: fi0,6,1,7 + v0-v4u1
            i += 1
        # c1 (batch-1 attention, yields 5-46): b1's last qk units on odd
        # 5-35, d0 on even 6-32, d1 partials on even 34-46, F1 on 41/43.
        st["xw"].extend(_d0_items(nc, P, C, aps, 0, st))
        d1p = _gen_d1P(nc, P, C, aps, st)
        d1f = _gen_d1F(nc, P, C, aps, st)
        j = 5
        for _ in c1:
            if j % 2 == 1 and j <= 35:
                _pull(f1, 1)                 # fi 2..11 remaining units
            elif j % 2 == 0 and j >= 34:
                _pull(d1p, 1)
            elif j in (41, 43):
                _pull(d1f, 1)                # F1 chunks 0,1 (pairs 3,4)
            j += 1
        _exhaust(f0)
        _exhaust(f1)
        _pop(st["xw"], 99)
        _exhaust(d1p)
        _exhaust(d1f)

    nc.compile()
    return nc


_NC_CACHE = {}


def _get_nc():
    if "nc" not in _NC_CACHE:
        _NC_CACHE["nc"] = build()
    return _NC_CACHE["nc"]


def make_in_maps(x, Wqkv, bqkv, Wo, bo):
    bf = ml_dtypes.bfloat16
    x = np.asarray(x, dtype=np.float32)
    Wqkv16 = np.ascontiguousarray(np.asarray(Wqkv, np.float32).astype(bf))
    bqkv = np.asarray(bqkv, dtype=np.float32)
    Wo16 = np.ascontiguousarray(np.asarray(Wo, np.float32).astype(bf))
    bo = np.asarray(bo, dtype=np.float32)
    bqc = np.ascontiguousarray(bqkv.reshape(F3 // 128, 128).T)
    bqv = np.ascontiguousarray(bqkv[2 * D:].reshape(1, D))
    bo_r = np.ascontiguousarray(bo.reshape(1, D))
    x16 = np.ascontiguousarray(x.astype(bf).transpose(0, 2, 1))
    in_maps = []
    for c in range(N_CORES):
        in_maps.append({
            "xs": np.ascontiguousarray(x16[c * BPC:(c + 1) * BPC]),
            "wqkv": Wqkv16,
            "bqc": bqc,
            "bqv": bqv,
            "wo": Wo16,
            "bo": bo_r,
        })
    return in_maps


def run(x, Wqkv, bqkv, Wo, bo, trace=False, **kw):
    nc = _get_nc()
    in_maps = make_in_maps(x, Wqkv, bqkv, Wo, bo)
    res = run_bass_kernel_spmd(nc, in_maps, list(range(N_CORES)), trace=trace, **kw)
    out = np.concatenate(
        [np.asarray(res.results[c]["ys"]).astype(np.float32) for c in range(N_CORES)],
        axis=0)
    return out, res


def kernel(x, Wqkv, bqkv, Wo, bo):
    out, _ = run(x, Wqkv, bqkv, Wo, bo)
    return out

